# revision 17
# baseline (speedup 1.0000x reference)
"""MoE-routed transformer encoder layer on 8 Trainium2 cores.

Routing (mean -> nearest center -> expert id) is computed on host; sentences
are dispatched to cores so that each core runs exactly one expert's weights
over its share of sentences (expert/data parallelism, no device collectives).
The device kernel is a dense encoder layer: QKV -> attention -> out-proj ->
LN1 -> FFN(gelu) -> LN2, computed in fp32 with fp32r (full-rate) matmuls.
"""

import numpy as np

H = 768
NH = 12
HD = 64
FF = 3072
S = 128
E = 4
EPS = 1e-12
NCORES = 8

PARAM_KEYS = [
    "wq", "wk", "wv", "wo", "bq", "bk", "bv", "bo",
    "ln1_g", "ln1_b", "w1", "b1", "w2", "b2", "ln2_g", "ln2_b",
]

_BUILD_CACHE = {}
LAST_RUN_WALL_NS = None
_SIM_GELU_IDENTITY = False  # test-only: CoreSim has no gelu table
_STAGE = 2  # debug: 0=x->out copy, 1=phase A only, 2=full
_SUB = 99  # debug sub-stage within phase A
_XT_F32 = False  # debug: xT in plain f32
_ATT_LVL = 4  # debug: 0=copy scores,1=+exp,2=+normalize,3=+transpose(full)


def _split_multi_waits(nc, mybir):
    # walrus in this env caps sync waits at 1 per CTRL-encoded instruction
    # (Drain); hoist extras onto single-wait InstDrain carriers inserted just
    # before the original. Compute/DMA instructions keep native multi-wait.
    for f in nc.m.functions:
        for b in f.blocks:
            insts = list(b.instructions)
            new, changed = [], False
            for inst in insts:
                si = inst.sync_info
                if (
                    isinstance(inst, mybir.InstDrain)
                    and si is not None
                    and len(si.on_wait) > 1
                ):
                    waits = list(si.on_wait)
                    for w in waits[:-1]:
                        d = mybir.InstDrain(
                            name=nc.get_next_instruction_name(), ins=[], outs=[]
                        )
                        d.engine = inst.engine
                        d.sync_info = mybir.SyncInfo(on_wait=[w], on_update=[])
                        nc.register_instruction(d)
                        new.append(d)
                    si.on_wait = [waits[-1]]
                    changed = True
                new.append(inst)
            if changed:
                b.instructions = new


def _build(nslot, use_mask):
    import concourse.bass as bass
    import concourse.mybir as mybir
    import concourse.tile as tile
    from concourse import bacc
    from concourse.masks import make_identity

    f32 = mybir.dt.float32
    f32r = mybir.dt.float32r
    AF = mybir.ActivationFunctionType
    ALU = mybir.AluOpType

    NS = nslot
    assert NS % 4 == 0
    G = NS // 4

    nc = bacc.Bacc("TRN2", target_bir_lowering=False, debug=False)

    x_d = nc.dram_tensor("x", [NS, S, H], f32, kind="ExternalInput").ap()
    mask_d = nc.dram_tensor("mask", [NS, S], f32, kind="ExternalInput").ap()
    wq_d = nc.dram_tensor("wq", [H, H], f32, kind="ExternalInput").ap()
    wk_d = nc.dram_tensor("wk", [H, H], f32, kind="ExternalInput").ap()
    wv_d = nc.dram_tensor("wv", [H, H], f32, kind="ExternalInput").ap()
    wo_d = nc.dram_tensor("wo", [H, H], f32, kind="ExternalInput").ap()
    bq_d = nc.dram_tensor("bq", [H], f32, kind="ExternalInput").ap()
    bk_d = nc.dram_tensor("bk", [H], f32, kind="ExternalInput").ap()
    bv_d = nc.dram_tensor("bv", [H], f32, kind="ExternalInput").ap()
    bo_d = nc.dram_tensor("bo", [H], f32, kind="ExternalInput").ap()
    g1_d = nc.dram_tensor("ln1_g", [H], f32, kind="ExternalInput").ap()
    b1l_d = nc.dram_tensor("ln1_b", [H], f32, kind="ExternalInput").ap()
    w1_d = nc.dram_tensor("w1", [H, FF], f32, kind="ExternalInput").ap()
    b1_d = nc.dram_tensor("b1", [FF], f32, kind="ExternalInput").ap()
    w2_d = nc.dram_tensor("w2", [FF, H], f32, kind="ExternalInput").ap()
    b2_d = nc.dram_tensor("b2", [H], f32, kind="ExternalInput").ap()
    g2_d = nc.dram_tensor("ln2_g", [H], f32, kind="ExternalInput").ap()
    b2l_d = nc.dram_tensor("ln2_b", [H], f32, kind="ExternalInput").ap()
    out_d = nc.dram_tensor("out", [NS, S, H], f32, kind="ExternalOutput").ap()

    x_sv = x_d.rearrange("n s h -> s n h")       # partition dim = sequence pos
    out_sv = out_d.rearrange("n s h -> s n h")

    def r(v):
        return v.bitcast(f32r)

    with tile.TileContext(nc) as tc:
        with (
            tc.tile_pool(name="const", bufs=1) as constp,
            tc.tile_pool(name="ybuf", bufs=1) as ybufp,
        ):
            ident = constp.tile([128, 128], f32)
            make_identity(nc, ident)
            eps_t = constp.tile([128, 1], f32)
            nc.vector.memset(eps_t, EPS)
            b1_sb = constp.tile([128, 24], f32)
            nc.gpsimd.dma_start(b1_sb, b1_d.rearrange("(o p) -> p o", p=128))

            def repl(pool, src, nm):
                t = pool.tile([128, H], f32, tag=nm, name=nm)
                bsrc = bass.AP(
                    tensor=src.tensor, offset=src.offset, ap=[[0, 128], [1, H]]
                )
                nc.gpsimd.dma_start(t, bsrc)
                return t

            b2_r = repl(constp, b2_d, "b2_r")
            g2_r = repl(constp, g2_d, "g2_r")
            b2l_r = repl(constp, b2l_d, "b2l_r")

            y_all = ybufp.tile([128, NS, H], f32)

            if _STAGE == 0:
                xt0 = ybufp.tile([128, NS, H], f32, tag="xt0", name="xt0")
                nc.sync.dma_start(xt0, x_sv)
                nc.sync.dma_start(out_sv, xt0)

            # ---------------- Phase A: attention + LN1 -> y_all ----------
            with (
                tc.tile_pool(name="pa", bufs=1) as pa,
                tc.tile_pool(name="pa2", bufs=2) as pa2,
                tc.tile_pool(name="psA_small", bufs=3, space="PSUM") as psAs,
                tc.tile_pool(name="psA_big", bufs=3, space="PSUM") as psAb,
                tc.tile_pool(name="psA_v", bufs=1, space="PSUM") as psAv,
            ):
                bq_sb = pa.tile([128, 6], f32, tag="bq_sb", name="bq_sb")
                nc.gpsimd.dma_start(bq_sb, bq_d.rearrange("(o p) -> p o", p=128))
                bk_sb = pa.tile([128, 6], f32, tag="bk_sb", name="bk_sb")
                nc.gpsimd.dma_start(bk_sb, bk_d.rearrange("(o p) -> p o", p=128))
                bv_r = repl(pa, bv_d, "bv_r")
                bo_r = repl(pa, bo_d, "bo_r")
                g1_r = repl(pa, g1_d, "g1_r")
                b1l_r = repl(pa, b1l_d, "b1l_r")
                for g in range(G if _STAGE >= 1 else 0):
                    s0 = g * 4
                    x_g = pa.tile([128, 4, H], f32, tag="x_g")
                    nc.sync.dma_start(x_g, x_sv[:, s0 : s0 + 4, :])
                    if use_mask:
                        mrep = pa.tile([128, 4, S], f32, tag="mrep")
                        src = bass.AP(
                            tensor=mask_d.tensor,
                            offset=s0 * S,
                            ap=[[0, 128], [S, 4], [1, S]],
                        )
                        nc.gpsimd.dma_start(mrep, src)

                    # x transposed: xT[p, c, si, s] = x[s, si, c*128+p]
                    xT = pa.tile([128, 6, 4, 128], f32 if _XT_F32 else f32r, tag="xT")
                    for si in range(4):
                        for c in range(6):
                            pt = psAs.tile([128, 128], f32, tag="pt")
                            nc.tensor.transpose(
                                pt, x_g[:, si, c * 128 : (c + 1) * 128], ident
                            )
                            nc.vector.tensor_copy(xT[:, c, si, :], pt)

                    if _SUB == 0:
                        ocp = pa.tile([128, 4, H], f32, tag="ocp", name="ocp")
                        nc.vector.tensor_copy(
                            ocp.rearrange("p n h -> p (n h)"),
                            xT.rearrange("p c n s -> p (c n s)").bitcast(f32),
                        )
                        nc.sync.dma_start(out_sv[:, s0 : s0 + 4, :], ocp)
                        continue

                    # qT/kT: weight-stationary over 4-sentence pack (N=512)
                    qT = pa.tile([128, 6, 4, 128], f32, tag="qT")
                    kT = pa.tile([128, 6, 4, 128], f32, tag="kT")
                    for w_dram, bias_sb, dstT in (
                        (wq_d, bq_sb, qT),
                        (wk_d, bk_sb, kT),
                    ):
                        w_sb = pa2.tile([128, 6, H], f32r, tag="wqkvo")
                        nc.sync.dma_start(
                            w_sb,
                            w_dram.rearrange("(ko p) m -> p ko m", p=128).bitcast(f32r),
                        )
                        for mc in range(6):
                            pq = psAb.tile([128, 512], f32, tag="pq")
                            for kc in range(6):
                                nc.tensor.matmul(
                                    pq,
                                    w_sb[:, kc, mc * 128 : (mc + 1) * 128],
                                    xT[:, kc, :, :],
                                    start=(kc == 0),
                                    stop=(kc == 5),
                                )
                            nc.scalar.activation(
                                dstT[:, mc, :, :],
                                pq,
                                AF.Identity,
                                bias=bias_sb[:, mc : mc + 1],
                                scale=1.0,
                            )

                    if _SUB == 1:
                        nc.sync.dma_start(
                            out_sv[:, s0 : s0 + 4, :],
                            qT.rearrange("p c n s -> p (c n s)")
                            .rearrange("p (n h) -> p n h", n=4),
                        )
                        continue

                    # v in natural layout [s, 768]
                    wv_sb = pa2.tile([128, 6, H], f32r, tag="wqkvo")
                    nc.sync.dma_start(
                        wv_sb,
                        wv_d.rearrange("(ko p) m -> p ko m", p=128).bitcast(f32r),
                    )
                    v_g = pa.tile([128, 4, H], f32, tag="v_g")
                    for si in range(4):
                        pv = psAv.tile([128, H], f32, tag="pv")
                        for kc in range(6):
                            nc.tensor.matmul(
                                pv[:, 0:512],
                                xT[:, kc, si, :],
                                wv_sb[:, kc, 0:512],
                                start=(kc == 0),
                                stop=(kc == 5),
                            )
                        for kc in range(6):
                            nc.tensor.matmul(
                                pv[:, 512:H],
                                xT[:, kc, si, :],
                                wv_sb[:, kc, 512:H],
                                start=(kc == 0),
                                stop=(kc == 5),
                            )
                        nc.vector.tensor_add(v_g[:, si, 0:512], pv[:, 0:512], bv_r[:, 0:512])
                        nc.vector.tensor_add(v_g[:, si, 512:H], pv[:, 512:H], bv_r[:, 512:H])

                    if _SUB == 2:
                        nc.sync.dma_start(out_sv[:, s0 : s0 + 4, :], v_g)
                        continue

                    # attention per sentence
                    ctxT = pa.tile([128, 6, 4, 128], f32r, tag="xT")  # reuse xT slot
                    for si in range(4):
                        attn = pa.tile([128, NH, S], f32, tag="attn")
                        sums = pa.tile([128, NH], f32, tag="sums")
                        for h in range(NH):
                            # one PSUM bank per head: a shared bank would be
                            # PE-written (next head) while read (this head),
                            # which is fatal on HW
                            psc = psAb.tile([128, 128], f32, tag="pq", name="psc")
                            nc.tensor.matmul(
                                psc,
                                qT[(h % 2) * 64 : (h % 2) * 64 + 64, h // 2, si, :],
                                kT[(h % 2) * 64 : (h % 2) * 64 + 64, h // 2, si, :],
                                start=True,
                                stop=True,
                            )
                            if _ATT_LVL == 0:
                                nc.vector.tensor_copy(attn[:, h, :], psc)
                            elif use_mask:
                                tmp = pa.tile([128, S], f32, tag="msk_tmp")
                                nc.vector.tensor_scalar_mul(tmp, psc, 0.125)
                                nc.vector.tensor_add(tmp, tmp, mrep[:, si, :])
                                nc.scalar.activation(
                                    attn[:, h, :], tmp, AF.Exp,
                                    bias=0.0, scale=1.0,
                                    accum_out=sums[:, h : h + 1],
                                )
                            else:
                                nc.scalar.activation(
                                    attn[:, h, :], psc, AF.Exp,
                                    bias=0.0, scale=0.125,
                                    accum_out=sums[:, h : h + 1],
                                )
                        if _ATT_LVL >= 2:
                            rs = pa.tile([128, NH], f32, tag="rs")
                            nc.vector.reciprocal(rs, sums)
                            for h in range(NH):
                                nc.vector.tensor_scalar_mul(
                                    attn[:, h, :], attn[:, h, :], rs[:, h : h + 1]
                                )
                        attnT = pa.tile([128, NH, S], f32, tag="attnT")
                        if _ATT_LVL >= 3:
                            for h in range(NH):
                                pt = psAs.tile([128, 128], f32, tag="pt")
                                nc.tensor.transpose(pt, attn[:, h, :], ident)
                                nc.vector.tensor_copy(attnT[:, h, :], pt)
                        else:
                            for h in range(NH):
                                nc.vector.tensor_copy(attnT[:, h, :], attn[:, h, :])
                        for hp in range(6):
                            pc = psAs.tile([128, 128], f32, tag="pt")
                            nc.tensor.matmul(
                                pc[0:64, :],
                                v_g[:, si, (2 * hp) * 64 : (2 * hp + 1) * 64],
                                attnT[:, 2 * hp, :],
                                start=True, stop=True,
                            )
                            nc.tensor.matmul(
                                pc[64:128, :],
                                v_g[:, si, (2 * hp + 1) * 64 : (2 * hp + 2) * 64],
                                attnT[:, 2 * hp + 1, :],
                                start=True, stop=True,
                            )
                            nc.vector.tensor_copy(ctxT[:, hp, si, :], pc)

                    if _SUB == 3:
                        nc.sync.dma_start(
                            out_sv[:, s0 : s0 + 4, :],
                            ctxT.rearrange("p c n s -> p (c n s)")
                            .rearrange("p (n h) -> p n h", n=4)
                            .bitcast(f32),
                        )
                        continue

                    # out-proj + bo + residual + LN1 -> y_all
                    wo_sb = pa2.tile([128, 6, H], f32r, tag="wqkvo")
                    nc.sync.dma_start(
                        wo_sb,
                        wo_d.rearrange("(ko p) m -> p ko m", p=128).bitcast(f32r),
                    )
                    for si in range(4):
                        po = psAv.tile([128, H], f32, tag="pv")
                        for kc in range(6):
                            nc.tensor.matmul(
                                po[:, 0:512],
                                ctxT[:, kc, si, :],
                                wo_sb[:, kc, 0:512],
                                start=(kc == 0), stop=(kc == 5),
                            )
                        for kc in range(6):
                            nc.tensor.matmul(
                                po[:, 512:H],
                                ctxT[:, kc, si, :],
                                wo_sb[:, kc, 512:H],
                                start=(kc == 0), stop=(kc == 5),
                            )
                        z = pa2.tile([128, H], f32, tag="z")
                        nc.vector.tensor_add(z[:, 0:512], po[:, 0:512], bo_r[:, 0:512])
                        nc.vector.tensor_add(z[:, 512:H], po[:, 512:H], bo_r[:, 512:H])
                        nc.vector.tensor_add(z, z, x_g[:, si, :])
                        # LN1
                        st = pa2.tile([128, 3, 6], f32, tag="st")
                        zv = z.rearrange("p (a b) -> p a b", a=3)
                        for i in range(3):
                            nc.vector.bn_stats(st[:, i, :], zv[:, i, :])
                        mv = pa2.tile([128, 2], f32, tag="mv")
                        nc.vector.bn_aggr(mv, st)
                        sd = pa2.tile([128, 1], f32, tag="sd")
                        nc.scalar.activation(sd, mv[:, 1:2], AF.Sqrt, bias=eps_t[:, 0:1], scale=1.0)
                        nc.vector.reciprocal(sd, sd)
                        yslot = y_all[:, s0 + si, :]
                        nc.vector.tensor_scalar(
                            yslot, z,
                            scalar1=mv[:, 0:1], scalar2=sd,
                            op0=ALU.subtract, op1=ALU.mult,
                        )
                        nc.vector.tensor_mul(yslot, yslot, g1_r)
                        nc.vector.tensor_add(yslot, yslot, b1l_r)

            if _STAGE == 1 and _SUB >= 4:
                nc.sync.dma_start(out_sv, y_all)
            # ---------------- Phase B: FFN + LN2 -> out ------------------
            with (
                tc.tile_pool(name="pb", bufs=1) as pb,
                tc.tile_pool(name="pb2", bufs=2) as pb2,
                tc.tile_pool(name="w2p", bufs=3) as w2p,
                tc.tile_pool(name="psB_a", bufs=1, space="PSUM") as psBa,
                tc.tile_pool(name="psB_g", bufs=2, space="PSUM") as psBg,
                tc.tile_pool(name="psB_t", bufs=1, space="PSUM") as psBt,
            ):
                for g in range(G if _STAGE >= 2 else 0):
                    s0 = g * 4
                    yT = pb.tile([128, 6, 4, 128], f32r, tag="yT")
                    for si in range(4):
                        for c in range(6):
                            pt = psBt.tile([128, 128], f32, tag="ptB")
                            nc.tensor.transpose(
                                pt, y_all[:, s0 + si, c * 128 : (c + 1) * 128], ident
                            )
                            nc.vector.tensor_copy(yT[:, c, si, :], pt)

                    # w1 + gelu for the whole group: gT [128, 24, 4*128]
                    gT = pb.tile([128, 24, 512], f32r, tag="gT")
                    gelu_fn = (
                        AF.Identity if _SIM_GELU_IDENTITY else AF.Gelu_apprx_tanh
                    )
                    for q in range(4):
                        w1q = pb2.tile([128, 6, 768], f32r, tag="w1q")
                        nc.sync.dma_start(
                            w1q,
                            w1_d.rearrange("(ko p) f -> p ko f", p=128)[
                                :, :, q * 768 : (q + 1) * 768
                            ].bitcast(f32r),
                        )
                        for fm in range(6):
                            pg = psBg.tile([128, 512], f32, tag="pg")
                            for kc in range(6):
                                nc.tensor.matmul(
                                    pg,
                                    w1q[:, kc, fm * 128 : (fm + 1) * 128],
                                    yT[:, kc, :, :],
                                    start=(kc == 0), stop=(kc == 5),
                                )
                            fg = q * 6 + fm
                            nc.scalar.activation(
                                gT[:, fg, :], pg, gelu_fn,
                                bias=b1_sb[:, fg : fg + 1], scale=1.0,
                            )

                    # w2: two column passes; each streams its w2 columns once
                    z2_all = pb.tile([128, 4, H], f32, tag="z2_all")
                    for (c0, c1) in ((0, 512), (512, H)):
                        pw2 = [
                            psBa.tile([128, 512], f32, tag=f"pw2_{i}", name=f"pw2_{i}")
                            for i in range(4)
                        ]
                        for kc in range(24):
                            w2c = w2p.tile([128, 512], f32r, tag="w2c")
                            nc.sync.dma_start(
                                w2c[:, : c1 - c0],
                                w2_d[kc * 128 : (kc + 1) * 128, c0:c1].bitcast(f32r),
                            )
                            for si in range(4):
                                nc.tensor.matmul(
                                    pw2[si][:, : c1 - c0],
                                    gT[:, kc, si * 128 : (si + 1) * 128],
                                    w2c[:, : c1 - c0],
                                    start=(kc == 0), stop=(kc == 23),
                                )
                        for si in range(4):
                            nc.vector.tensor_add(
                                z2_all[:, si, c0:c1],
                                pw2[si][:, : c1 - c0],
                                b2_r[:, c0:c1],
                            )

                    o_g = pb2.tile([128, 4, H], f32, tag="o_g")
                    for si in range(4):
                        z2 = z2_all[:, si, :]
                        nc.vector.tensor_add(z2, z2, y_all[:, s0 + si, :])
                        st = pb2.tile([128, 3, 6], f32, tag="stB")
                        z2v = z2.rearrange("p (a b) -> p a b", a=3)
                        for i in range(3):
                            nc.vector.bn_stats(st[:, i, :], z2v[:, i, :])
                        mv = pb2.tile([128, 2], f32, tag="mvB")
                        nc.vector.bn_aggr(mv, st)
                        sd = pb2.tile([128, 1], f32, tag="sdB")
                        nc.scalar.activation(sd, mv[:, 1:2], AF.Sqrt, bias=eps_t[:, 0:1], scale=1.0)
                        nc.vector.reciprocal(sd, sd)
                        oslot = o_g[:, si, :]
                        nc.vector.tensor_scalar(
                            oslot, z2,
                            scalar1=mv[:, 0:1], scalar2=sd,
                            op0=ALU.subtract, op1=ALU.mult,
                        )
                        nc.vector.tensor_mul(oslot, oslot, g2_r)
                        nc.vector.tensor_add(oslot, oslot, b2l_r)
                    nc.sync.dma_start(out_sv[:, s0 : s0 + 4, :], o_g)

    nc.compile()
    return nc


def _route_and_assign(hidden_states, centers):
    hp = hidden_states.mean(axis=1)  # [B, H]
    d2 = (
        (hp * hp).sum(-1, keepdims=True)
        - 2.0 * hp @ centers.T
        + (centers * centers).sum(-1)[None, :]
    )
    eid = np.argmin(d2, axis=1)  # [B]
    B = eid.shape[0]
    counts = np.bincount(eid, minlength=E)
    active = [e for e in range(E) if counts[e] > 0]
    # apportion cores to active experts proportionally (min 1 each)
    cores_e = {e: 1 for e in active}
    rem = NCORES - len(active)
    if rem > 0:
        quota = {e: counts[e] * NCORES / B for e in active}
        frac = {e: quota[e] - 1 for e in active}
        order = sorted(active, key=lambda e: -frac[e])
        whole = {e: max(0, int(np.floor(frac[e]))) for e in active}
        used = sum(whole.values())
        while used > rem:  # trim if overflow
            for e in sorted(active, key=lambda e: -whole[e]):
                if used <= rem:
                    break
                if whole[e] > 0:
                    whole[e] -= 1
                    used -= 1
        for e in active:
            cores_e[e] += whole[e]
        rem -= used
        i = 0
        frac_order = sorted(active, key=lambda e: -(frac[e] - whole[e]))
        while rem > 0:
            cores_e[frac_order[i % len(frac_order)]] += 1
            rem -= 1
            i += 1
    # assign sentences of each expert round-robin over its cores
    assign = [[] for _ in range(NCORES)]  # core -> list of batch idx
    core_expert = [active[0] if active else 0] * NCORES
    next_core = 0
    for e in active:
        ncr = cores_e[e]
        idxs = np.nonzero(eid == e)[0]
        chunks = np.array_split(idxs, ncr)
        for ch in chunks:
            assign[next_core] = list(ch)
            core_expert[next_core] = e
            next_core += 1
    max_load = max(len(a) for a in assign)
    nslot = max(4, int(np.ceil(max_load / 4.0)) * 4)
    return assign, core_expert, nslot


def kernel(**inputs):
    global LAST_RUN_WALL_NS
    import time

    from concourse.bass_utils import run_bass_kernel_spmd

    inputs = {k: np.ascontiguousarray(np.asarray(v)) for k, v in inputs.items()}
    hs = inputs["hidden_states"].astype(np.float32, copy=False)
    am = inputs["attention_mask"].astype(np.float32, copy=False)
    centers = inputs["centers"].astype(np.float32, copy=False)
    B = hs.shape[0]

    assign, core_expert, nslot = _route_and_assign(hs, centers)
    use_mask = bool(np.any(am != 0.0))

    key = (nslot, use_mask)
    if key not in _BUILD_CACHE:
        _BUILD_CACHE[key] = _build(nslot, use_mask)
    nc = _BUILD_CACHE[key]

    in_maps = []
    for c in range(NCORES):
        e = core_expert[c]
        idxs = assign[c]
        x = np.zeros((nslot, S, H), np.float32)
        m = np.zeros((nslot, S), np.float32)
        for j, b in enumerate(idxs):
            x[j] = hs[b]
            m[j] = am[b]
        im = {"x": x, "mask": m}
        for k in PARAM_KEYS:
            im[k] = np.ascontiguousarray(inputs[k][e])
        in_maps.append(im)

    t0 = time.perf_counter_ns()
    res = run_bass_kernel_spmd(nc, in_maps, core_ids=list(range(NCORES)))
    LAST_RUN_WALL_NS = time.perf_counter_ns() - t0

    out = np.zeros((B, S, H), np.float32)
    for c in range(NCORES):
        oc = res.results[c]["out"]
        for j, b in enumerate(assign[c]):
            out[b] = oc[j]
    return out
